# revision 6
# baseline (speedup 1.0000x reference)
"""Distributed Trainium2 kernel for nn_AddAttention_154618823089.

Computation (see reference):
    q = rope(bf16(hidden @ Wq.T)); k = rope(bf16(hidden @ Wk.T))
    o[b,l] = sum_{j<=l} exp(q_l . k_j / sqrt(DIM))          (no softmax norm)
    out = relu(o @ fc1_w.T + fc1_b) @ fc2_w.T + fc2_b

Sharding v2: 4 cores per batch (core c -> batch c//4), each core owns the
strided row set {4m + c%4} of its batch (1024 rows).  Striding keeps the
causal workload identical on every core; batch-splitting means each core
only ever needs ITS batch's k: the k exchange is ONE AllGather with two
disjoint 4-rank groups (wire 3MB/rank vs 7MB for the old 8-rank x2 scheme)
and the gathered-k SBUF reload is 4MB instead of 8MB.

fp8 pipeline (rel err ~3e-3 vs the 2e-2 gate):
  - DoubleRow fp8 projections, RoPE in bf16 on vector, fp8 q_rope/k_rope
  - one bounce (1MB) -> one AllGather -> kt tile [128, t8, kb8, r4, jj128]
    loaded with 4 big DMAs (one per source rank, 8KB/partition contiguous
    reads) spread over 4 engine queues
  - a dependency-free 512B AllGather issued first pulls the ~30us ncfw
    startup barrier to the front of the NEFF
  - causal score blocks [128q x 512k], exp fused with row-sum via
    accum_out; MLP per subtile; bf16 output (cast to f32 on host)
"""

import sys
import types

import numpy as np
from ml_dtypes import bfloat16, float8_e4m3

import concourse.bacc as bacc
import concourse.bass as bass
import concourse.mybir as mybir
import concourse.tile as tile
from concourse.bass_utils import run_bass_kernel_spmd


def _install_ntff_hook():
    """The container's antenv lacks axon_hooks; provide it so trace=True can
    capture NTFF profiles (exec_time_ns) through the axon PJRT library."""
    if "antenv.axon_hooks" in sys.modules:
        return
    try:
        sys.path.insert(0, "/root/.axon_site/trn_agent_boot")
        import trn_boot

        mod = types.ModuleType("antenv.axon_hooks")
        _h = {"hook": None}
        mod.set_axon_ntff_profile_hook = lambda h: _h.__setitem__("hook", h)
        mod.get_axon_ntff_profile_hook = lambda: _h["hook"]
        sys.modules["antenv.axon_hooks"] = mod
        import antenv

        antenv.axon_hooks = mod
        mod.set_axon_ntff_profile_hook(
            trn_boot._ntff_profile_via_ctypes("/opt/axon/libaxon_pjrt.so"))
    except Exception:
        pass


_install_ntff_hook()

B, L, DIM, INNER = 2, 4096, 1024, 16
ROPE_BASE = 32.0
NCORES = 8
GSIZE = 4              # cores per batch / replica group size
RLOC = L // GSIZE      # local q/k rows per core (1024, one batch)
NSUB = RLOC // 128     # q subtiles per core (8)
NDT = DIM // 128       # d tiles (8)
NDP = NDT // 2         # DoubleRow d-tile pairs (4)
SCALE = 1.0 / float(np.sqrt(DIM))
MASK_NEG = -1.0e6
CHUNK = 3              # psum banks per score chunk
F32 = mybir.dt.float32
BF16 = mybir.dt.bfloat16
F8 = mybir.dt.float8e4
DR = mybir.MatmulPerfMode.DoubleRow

_NC_CACHE = {}


def _build_nc():
    nc = bacc.Bacc("TRN2", target_bir_lowering=False, debug=False,
                   num_devices=NCORES, num_swdge_queues=4)

    hT = nc.dram_tensor("hT", [DIM, RLOC], F8, kind="ExternalInput")
    wqT = nc.dram_tensor("wqT", [DIM, DIM], F8, kind="ExternalInput")
    wkT = nc.dram_tensor("wkT", [DIM, DIM], F8, kind="ExternalInput")
    cosh = nc.dram_tensor("cosh", [DIM // 2, RLOC], BF16, kind="ExternalInput")
    sinh = nc.dram_tensor("sinh", [DIM // 2, RLOC], BF16, kind="ExternalInput")
    mask_d = nc.dram_tensor("mask", [128, 512], F32, kind="ExternalInput")
    w1b_d = nc.dram_tensor("w1b", [128, 32], F32, kind="ExternalInput")
    b1b_d = nc.dram_tensor("b1b", [128, 32], F32, kind="ExternalInput")
    w2aug = nc.dram_tensor("w2aug", [INNER + 1, DIM], BF16, kind="ExternalInput")
    onesrow = nc.dram_tensor("onesrow", [1, RLOC], BF16, kind="ExternalInput")
    out_d = nc.dram_tensor("out", [RLOC, DIM], BF16, kind="ExternalOutput")

    kb_bounce = nc.dram_tensor("kTb", [128, NDT, RLOC], F8)
    G = nc.dram_tensor("G", [GSIZE * 128, NDT, RLOC], F8)
    dum_in = nc.dram_tensor("dumin", [1, 512], F8)
    dum_out = nc.dram_tensor("dumout", [NCORES, 512], F8, addr_space="Shared")

    groups = [[0, 1, 2, 3], [4, 5, 6, 7]]
    all_grp = [list(range(NCORES))]

    with tile.TileContext(nc) as tc:
        with (
            tc.tile_pool(name="big", bufs=1) as big,
            tc.tile_pool(name="tmp", bufs=2) as tmp,
            tc.tile_pool(name="stg", bufs=2) as stg,
            tc.tile_pool(name="rsp", bufs=3) as rsp,
            tc.tile_pool(name="obp", bufs=4) as obp,
            tc.tile_pool(name="ps", bufs=7, space="PSUM") as pps,
            tc.tile_pool(name="po", bufs=1, space="PSUM") as ppo,
        ):
            # dependency-free tiny AllGather first: starts the ncfw startup
            # barrier at t~0 instead of when the real gather triggers
            nc.gpsimd.collective_compute(
                "AllGather", mybir.AluOpType.bypass, replica_groups=all_grp,
                ins=[dum_in.ap().opt()], outs=[dum_out.ap().opt()])

            # ---- inputs -> SBUF as DoubleRow pair tiles, spread on queues --
            h_r = hT.rearrange("(dp k2 p) r -> dp p k2 r", dp=NDP, k2=2, p=128)
            wk_r = wkT.rearrange("(dp k2 p) r -> dp p k2 r",
                                 dp=NDP, k2=2, p=128)
            wq_r = wqT.rearrange("(dp k2 p) r -> dp p k2 r",
                                 dp=NDP, k2=2, p=128)
            h_t, wk_t, wq_t = [], [], []
            for dp in range(NDP):
                th = big.tile([128, 2, RLOC], F8, tag=f"h{dp}", name=f"h{dp}")
                nc.sync.dma_start(th[:], h_r[dp])
                h_t.append(th)
                tw = big.tile([128, 2, DIM], F8, tag=f"wk{dp}", name=f"wk{dp}")
                nc.scalar.dma_start(tw[:], wk_r[dp])
                wk_t.append(tw)
            cos_t, sin_t = [], []
            for ci in range(NDT // 2):
                tc_ = big.tile([128, RLOC], BF16, tag=f"cos{ci}",
                               name=f"cos{ci}")
                nc.sync.dma_start(tc_[:], cosh[128 * ci:128 * (ci + 1), :])
                cos_t.append(tc_)
                ts_ = big.tile([128, RLOC], BF16, tag=f"sin{ci}",
                               name=f"sin{ci}")
                nc.gpsimd.dma_start(ts_[:], sinh[128 * ci:128 * (ci + 1), :])
                sin_t.append(ts_)
            for dp in range(NDP):
                tw = big.tile([128, 2, DIM], F8, tag=f"wq{dp}", name=f"wq{dp}")
                nc.sync.dma_start(tw[:], wq_r[dp])
                wq_t.append(tw)
            mask_sb = big.tile([128, 512], F32, tag="mask")
            nc.scalar.dma_start(mask_sb[:], mask_d[:])
            w1b_sb = big.tile([128, 32], F32, tag="w1b")
            nc.scalar.dma_start(w1b_sb[:], w1b_d[:])
            b1b_sb = big.tile([128, 32], F32, tag="b1b")
            nc.scalar.dma_start(b1b_sb[:], b1b_d[:])
            w2_sb = big.tile([INNER + 1, DIM], BF16, tag="w2")
            nc.scalar.dma_start(w2_sb[:], w2aug[:])
            z_aug = big.tile([INNER + 1, RLOC], BF16, tag="zaug")
            nc.scalar.dma_start(z_aug[INNER:INNER + 1, :], onesrow[:])

            def project_half(w_t, proj, rt, bounce=False):
                """proj[:, :, 512rt:512rt+512] = fp8(rope(W @ h^T)).
                DoubleRow fp8 matmuls -> psum f32 -> bf16 staging (scalar)
                -> rope on vector -> fp8 slots (dt, dt+4); do-order
                interleaves the (dt, dt+4) halves so RoPE pairs complete
                (and optionally bounce to DRAM) right behind PE."""
                cols = slice(512 * rt, 512 * (rt + 1))
                pbf = stg.tile([128, NDT, 512], BF16, tag="pbf",
                               name=f"pbf{rt}")

                def rope_pair(dt):
                    cm = cos_t[dt][:, cols]
                    sm = sin_t[dt][:, cols]
                    lo = pbf[:, dt, :]
                    hi = pbf[:, dt + NDT // 2, :]
                    ta = tmp.tile([128, 512], BF16, tag="ta", name="ta")
                    tb = tmp.tile([128, 512], BF16, tag="tb", name="tb")
                    td = tmp.tile([128, 512], BF16, tag="td", name="td")
                    nc.vector.tensor_mul(ta[:], lo, cm)
                    nc.vector.tensor_mul(tb[:], lo, sm)
                    nc.vector.tensor_mul(td[:], hi, sm)
                    nc.vector.tensor_sub(proj[:, dt, cols], ta[:], td[:])
                    nc.vector.tensor_mul(ta[:], hi, cm)
                    nc.vector.tensor_add(proj[:, dt + NDT // 2, cols],
                                         ta[:], tb[:])
                    if bounce:
                        # both rope slots of the pair in one strided dma
                        eng = nc.sync if dt % 2 else nc.scalar
                        eng.dma_start(
                            kb_bounce[:, dt::NDT // 2, cols],
                            proj[:, dt::NDT // 2, cols])

                order = [x for pair in zip(range(NDT // 2),
                                           range(NDT // 2, NDT))
                         for x in pair]            # 0,4,1,5,2,6,3,7
                for do in order:
                    ps = pps.tile([128, 512], F32, tag="ps",
                                  name=f"psp{rt}{do}")
                    for dp in range(NDP):
                        nc.tensor.matmul(
                            ps[:], w_t[dp][:, :, 128 * do:128 * (do + 1)],
                            h_t[dp][:, :, cols],
                            start=(dp == 0), stop=(dp == NDP - 1),
                            perf_mode=DR,
                        )
                    # f32 psum -> bf16 staging for rope (reference casts
                    # q/k to bf16 here); scalar ACT keeps vector free for
                    # rope and unblocks psum banks for the next matmuls
                    nc.scalar.activation(pbf[:, do, :], ps[:],
                                         mybir.ActivationFunctionType.Copy)
                    if do >= NDT // 2:
                        rope_pair(do - NDT // 2)

            # ---- k: project+rope+bounce both halves, then ONE AllGather
            # (two disjoint 4-rank groups); q projects during the gather --
            k_rope = big.tile([128, NDT, RLOC], F8, tag="krope")
            project_half(wk_t, k_rope, 0, bounce=True)
            project_half(wk_t, k_rope, 1, bounce=True)
            nc.gpsimd.collective_compute(
                "AllGather", mybir.AluOpType.bypass, replica_groups=groups,
                ins=[kb_bounce.ap().opt()], outs=[G.ap().opt()])

            q_rope = big.tile([128, NDT, RLOC], F8, tag="qrope")
            project_half(wq_t, q_rope, 0)
            project_half(wq_t, q_rope, 1)

            # ---- gathered-K -> SBUF: kt [128 dpart, t4, kb8, r4, jj128]
            # x2 t-halves (separate tiles so dp 0-1 score matmuls start as
            # soon as the low half lands), one 512KB dma per (rank, half)
            # (4KB/partition contiguous reads), spread over the 3 dma-
            # capable queues ---------------------------------------------
            g_r = G.rearrange("(r p) t (kb jj) -> r p t kb jj",
                              r=GSIZE, p=128, kb=NSUB, jj=128)
            _kteng = [nc.sync, nc.scalar, nc.gpsimd]
            kts = []
            for th in range(2):
                ktt = big.tile([128, NDT // 2, NSUB, GSIZE, 128], F8,
                               tag=f"kt{th}")
                for r in range(GSIZE):
                    eng = _kteng[(4 * th + r) % 3]
                    eng.dma_start(ktt[:, :, :, r, :],
                                  g_r[r, :, 4 * th:4 * (th + 1), :, :])
                kts.append(ktt)

            o_sb = big.tile([128, NSUB], F32, tag="o")

            def mlp_sub(s):
                # o_sb[p, s] is local row 128s + p.
                # z[row, n] = relu(o[row]*w1[n] + b1[n]) with o as a
                # per-partition scalar, DVE-transposed into z_aug[n, row],
                # then out rows = z_aug.T @ w2aug.
                zrow = tmp.tile([128, 32], F32, tag="zr", name=f"zr{s}")
                nc.vector.tensor_scalar_mul(zrow[:], w1b_sb[:],
                                            o_sb[:, s:s + 1])
                nc.vector.tensor_add(zrow[:], zrow[:], b1b_sb[:])
                zrb = tmp.tile([128, 32], BF16, tag="zrb", name=f"zrb{s}")
                nc.vector.tensor_scalar_max(zrb[:], zrow[:], 0.0)
                zts = tmp.tile([32, 128], BF16, tag="zts", name=f"zts{s}")
                for g in range(4):
                    nc.vector.transpose(zts[0:32, 32 * g:32 * (g + 1)],
                                        zrb[32 * g:32 * (g + 1), :])
                nc.vector.tensor_copy(z_aug[0:INNER, 128 * s:128 * (s + 1)],
                                      zts[0:INNER, :])
                row0 = 128 * s
                ob = obp.tile([128, DIM], BF16, tag="ob", name=f"ob{s}")
                for hh in range(2):
                    po = ppo.tile([128, 512], F32, tag="po",
                                  name=f"po{s}{hh}")
                    nc.tensor.matmul(po[:],
                                     z_aug[:, 128 * s:128 * (s + 1)],
                                     w2_sb[:, 512 * hh:512 * (hh + 1)],
                                     start=True, stop=True)
                    nc.vector.tensor_copy(ob[:, 512 * hh:512 * (hh + 1)],
                                          po[:])
                eng = nc.gpsimd if s % 2 else nc.sync
                eng.dma_start(out_d[row0:row0 + 128, :], ob[:])

            # ---- causal scores: s in 0..7, k blocks kb<=s, chunks of 3
            # psum banks; exp fused with row-sum via accum_out ------------
            for s in range(NSUB):
                rs_t = rsp.tile([128, NSUB], F32, tag=f"rs{s % 3}",
                                name=f"rs{s}")
                blocks = list(range(s + 1))
                for c0 in range(0, len(blocks), CHUNK):
                    chunk = blocks[c0:c0 + CHUNK]
                    psl = [pps.tile([128, 512], F32, tag="ps",
                                    name=f"ps{s}{c0}_{i}")
                           for i in range(len(chunk))]
                    for dp in range(NDP):
                        lhsT = q_rope[:, 2 * dp:2 * dp + 2,
                                      128 * s:128 * (s + 1)]
                        dpl = 2 * (dp % 2)
                        for kb, ps in zip(chunk, psl):
                            nc.tensor.matmul(
                                ps[:], lhsT,
                                kts[dp // 2][:, dpl:dpl + 2, kb, :, :],
                                start=(dp == 0), stop=(dp == NDP - 1),
                                perf_mode=DR,
                            )
                    for kb, ps in zip(chunk, psl):
                        if kb == s:
                            nc.vector.tensor_add(ps[:], ps[:], mask_sb[:])
                        nc.scalar.activation(
                            ps[:], ps[:],
                            mybir.ActivationFunctionType.Exp,
                            scale=SCALE,
                            accum_out=rs_t[:, kb:kb + 1],
                        )
                nc.vector.reduce_sum(o_sb[:, s:s + 1], rs_t[:, 0:s + 1],
                                     axis=mybir.AxisListType.X)
                mlp_sub(s)

    nc.compile()
    return nc


def get_nc():
    if "nc" not in _NC_CACHE:
        _NC_CACHE["nc"] = _build_nc()
    return _NC_CACHE["nc"]


def make_in_maps(hidden_states, Wq, Wk, fc1_w, fc1_b, fc2_w, fc2_b):
    hidden_states = np.asarray(hidden_states, dtype=np.float32)
    Wq = np.asarray(Wq, dtype=np.float32)
    Wk = np.asarray(Wk, dtype=np.float32)
    fc1_w = np.asarray(fc1_w, dtype=np.float32)
    fc1_b = np.asarray(fc1_b, dtype=np.float32)
    fc2_w = np.asarray(fc2_w, dtype=np.float32)
    fc2_b = np.asarray(fc2_b, dtype=np.float32)

    wqT = np.ascontiguousarray(Wq.T).astype(float8_e4m3)
    wkT = np.ascontiguousarray(Wk.T).astype(float8_e4m3)
    w1b = np.zeros((128, 32), dtype=np.float32)
    w1b[:, 0:INNER] = fc1_w.reshape(1, INNER)
    b1b = np.zeros((128, 32), dtype=np.float32)
    b1b[:, 0:INNER] = fc1_b.reshape(1, INNER)
    w2aug = np.concatenate([fc2_w.T, fc2_b[None, :]], axis=0).astype(bfloat16)

    inv_freq = ROPE_BASE ** (-np.arange(0, DIM, 2, dtype=np.float32) / DIM)

    in_maps = []
    for c in range(NCORES):
        b, j = c // GSIZE, c % GSIZE
        rows = np.arange(RLOC) * GSIZE + j           # global rows of batch b
        hT = np.ascontiguousarray(
            hidden_states[b, rows, :].T).astype(float8_e4m3)  # [DIM, RLOC]
        ang = rows[:, None].astype(np.float32) * inv_freq[None, :]  # [RLOC,512]
        cosh = np.ascontiguousarray(np.cos(ang).T).astype(bfloat16)
        sinh = np.ascontiguousarray(np.sin(ang).T).astype(bfloat16)
        # mask[p, r*128+t]: allow k col (rank r, t) for q row p iff
        # 4t + r <= 4p + j  (boundary subtile kb==s; same for every s)
        p = np.arange(128)[:, None, None]
        r = np.arange(GSIZE)[None, :, None]
        t = np.arange(128)[None, None, :]
        allow = (GSIZE * t + r) <= (GSIZE * p + j)
        mask = np.where(allow, 0.0, MASK_NEG).astype(np.float32)
        in_maps.append({
            "hT": hT,
            "wqT": wqT, "wkT": wkT,
            "cosh": cosh, "sinh": sinh,
            "mask": np.ascontiguousarray(mask.reshape(128, 512)),
            "w1b": w1b, "b1b": b1b, "w2aug": w2aug,
            "onesrow": np.ones((1, RLOC), dtype=bfloat16),
        })
    return in_maps


def assemble_output(results):
    out = np.empty((B, L, DIM), dtype=np.float32)
    for c in range(NCORES):
        b, j = c // GSIZE, c % GSIZE
        out[b, j::GSIZE, :] = results[c]["out"].astype(np.float32)
    return out


def run(trace=False, **inputs):
    nc = get_nc()
    in_maps = make_in_maps(**inputs)
    res = run_bass_kernel_spmd(nc, in_maps, core_ids=list(range(NCORES)),
                               trace=trace)
    return assemble_output(res.results), res


def kernel(**inputs) -> np.ndarray:
    out, _ = run(trace=False, **inputs)
    return out
